# revision 17
# baseline (speedup 1.0000x reference)
"""Trainium2 kernel for nn_NonsharedPatchEmbed: 196 independent Linear(768->768)
applied per patch of a patchified [64, 3, 224, 224] image batch.

  out[b, p, o] = sum_i patches[b, p, i] * W[p, o, i] + b[p, o]

Strategy: shard the 196-patch axis across the 8 NeuronCores (25 patches per
core, padded to 200). Per patch this is a [64, 768] x [768, 768]^T GEMM with
the full batch as the stationary operand; the per-patch W (the dominant
traffic, 462 MB total) streams through the tensor engine exactly once.

Host-side work is layout only: patchify x, pre-transpose W to W^T, and split
the bias into a bf16 hi+lo pair (added exactly via a K=2 ones-matmul into the
same PSUM accumulation group).

Layouts per core (mode A, W moving):
  aT  [128, 25, 6, 64]  f32   aT[i, p, c, b] = patches[b, 25k+p, 128c+i]
  Wt  [25, 128, 6, 768] f32   Wt[p, i, c, o] = W[25k+p, o, 128c+i]
  bhl [2, 25, 768]      bf16  bias hi/lo split
  out [25, 64, 768]     f32

Mode B (W stationary): same inputs plus bias_pp [128, 25, 6] f32; per
(patch, o_chunk) accumulates psum [128, 64] over 6 i-chunks with W^T blocks
as lhsT; bias added per-partition during the PSUM->SBUF copy.
"""

import os
import numpy as np
import ml_dtypes

import concourse.bass as bass
import concourse.tile as tile
import concourse.mybir as mybir
from concourse import bacc
from concourse.bass_utils import run_bass_kernel_spmd

f32 = mybir.dt.float32
bf16 = mybir.dt.bfloat16

N_CORES = 8
B = 64
D = 768
NP = 196
PPC = 25          # patches per core (8*25 = 200, last 4 padded)
NCHUNK = 6        # 768 / 128

LAST_RESULTS = None  # BassKernelResults of the most recent run (for test.py)

_NC_CACHE = {}


def _build(mode):
    if mode == "A2":
        return _build_a2()
    if mode == "C":
        return _build_c()
    if mode == "D":
        return _build_d()
    if mode == "E":
        return _build_e()
    nc = bacc.Bacc()
    aT = nc.declare_dram_parameter("aT", [128, PPC, NCHUNK, B], f32, isOutput=False)
    Wt = nc.declare_dram_parameter("Wt", [PPC, 128, NCHUNK, D], f32, isOutput=False)
    if mode == "A":
        bhl = nc.declare_dram_parameter("bhl", [2, PPC, D], bf16, isOutput=False)
        out = nc.declare_dram_parameter("out", [PPC, B, D], f32, isOutput=True)
    else:
        bpp = nc.declare_dram_parameter("bpp", [128, PPC, NCHUNK], f32, isOutput=False)
        out = nc.declare_dram_parameter("out", [PPC, 128, NCHUNK, B], f32, isOutput=True)

    with tile.TileContext(nc) as tc:
        with (
            tc.tile_pool(name="const", bufs=1) as cpool,
            tc.tile_pool(name="w", bufs=3) as wpool,
            tc.tile_pool(name="o", bufs=3) as opool,
            tc.tile_pool(name="ps", bufs=4, space="PSUM") as pspool,
        ):
            ta = cpool.tile([128, PPC, NCHUNK, B], f32)
            nc.sync.dma_start(ta[:], aT[:])
            if mode == "A":
                ones = cpool.tile([2, B], bf16)
                nc.vector.memset(ones[:], 1.0)
                tb = cpool.tile([2, PPC, D], bf16)
                nc.sync.dma_start(tb[:], bhl[:])
            else:
                tbias = cpool.tile([128, PPC, NCHUNK], f32)
                nc.sync.dma_start(tbias[:], bpp[:])

            for p in range(PPC):
                wt = wpool.tile([128, NCHUNK, D], f32)
                nc.sync.dma_start(wt[:], Wt[p])

                if mode == "A":
                    pt = pspool.tile([B, D], f32)
                    slices = [(0, 512), (512, 768)]
                    for (o0, o1) in slices:
                        nc.tensor.matmul(
                            pt[:, o0:o1], ones[:], tb[:, p, o0:o1],
                            start=True, stop=False,
                        )
                    for c in range(NCHUNK):
                        for (o0, o1) in slices:
                            nc.tensor.matmul(
                                pt[:, o0:o1], ta[:, p, c, :], wt[:, c, o0:o1],
                                start=False, stop=(c == NCHUNK - 1),
                            )
                    ob = opool.tile([B, D], f32)
                    nc.vector.tensor_copy(ob[:], pt[:])
                    nc.sync.dma_start(out[p], ob[:])
                else:
                    ob = opool.tile([128, NCHUNK, B], f32)
                    for oc in range(NCHUNK):
                        pt = pspool.tile([128, B], f32)
                        for c in range(NCHUNK):
                            nc.tensor.matmul(
                                pt[:], wt[:, c, oc * 128:(oc + 1) * 128],
                                ta[:, p, c, :],
                                start=(c == 0), stop=(c == NCHUNK - 1),
                            )
                        nc.vector.tensor_scalar_add(
                            ob[:, oc, :], pt[:], tbias[:, p, oc:oc + 1]
                        )
                    nc.sync.dma_start(out[p], ob[:])

    nc.finalize()
    return nc


def _build_a2():
    """Mode A with: per-patch aT loads (fast start) and bias added on DVE via
    partition-broadcast during the PSUM->SBUF copy (no PE bias matmuls)."""
    nc = bacc.Bacc()
    aT = nc.declare_dram_parameter("aT", [PPC, 128, NCHUNK, B], f32, isOutput=False)
    Wt = nc.declare_dram_parameter("Wt", [PPC, 128, NCHUNK, D], f32, isOutput=False)
    bhl = nc.declare_dram_parameter("bhl", [2, PPC, D], bf16, isOutput=False)
    out = nc.declare_dram_parameter("out", [PPC, B, D], f32, isOutput=True)

    with tile.TileContext(nc) as tc:
        with (
            tc.tile_pool(name="const", bufs=1) as cpool,
            tc.tile_pool(name="w", bufs=3) as wpool,
            tc.tile_pool(name="o", bufs=3) as opool,
            tc.tile_pool(name="ps", bufs=4, space="PSUM") as pspool,
        ):
            ones = cpool.tile([2, B], bf16)
            nc.vector.memset(ones[:], 1.0)
            tb = cpool.tile([2, PPC, D], bf16)
            nc.sync.dma_start(tb[:], bhl[:])
            ta = cpool.tile([128, PPC, NCHUNK, B], f32)
            for p in range(PPC):
                nc.sync.dma_start(ta[:, p], aT[p])

            slices = [(0, 512), (512, 768)]
            for p in range(PPC):
                wt = wpool.tile([128, NCHUNK, D], f32)
                nc.sync.dma_start(wt[:], Wt[p])
                pt = pspool.tile([B, D], f32)
                for (o0, o1) in slices:
                    nc.tensor.matmul(
                        pt[:, o0:o1], ones[:], tb[:, p, o0:o1],
                        start=True, stop=False,
                    )
                for c in range(NCHUNK):
                    for (o0, o1) in slices:
                        nc.tensor.matmul(
                            pt[:, o0:o1], ta[:, p, c, :], wt[:, c, o0:o1],
                            start=False, stop=(c == NCHUNK - 1),
                        )
                ob = opool.tile([B, D], f32)
                nc.vector.tensor_copy(ob[:], pt[:])
                nc.sync.dma_start(out[p], ob[:])

    nc.finalize()
    return nc


def _build_c():
    """Col-tiled patch pairs: two patches share the 128x128 array (cols 0-63 /
    64-127), each streaming its own W. Full-partition PSUM + output DMAs."""
    nc = bacc.Bacc()
    NPAIR = PPC // 2           # 12 pairs + 1 leftover patch
    aT = nc.declare_dram_parameter("aT", [PPC, 128, NCHUNK, B], f32, isOutput=False)
    Wt = nc.declare_dram_parameter("Wt", [PPC, 128, NCHUNK, D], f32, isOutput=False)
    bhl = nc.declare_dram_parameter("bhl", [2, PPC, D], bf16, isOutput=False)
    outp = nc.declare_dram_parameter("outp", [NPAIR, 2 * B, D], f32, isOutput=True)
    outl = nc.declare_dram_parameter("outl", [B, D], f32, isOutput=True)

    with tile.TileContext(nc) as tc:
        with (
            tc.tile_pool(name="const", bufs=1) as cpool,
            tc.tile_pool(name="w", bufs=2) as wpool,
            tc.tile_pool(name="o", bufs=3) as opool,
            tc.tile_pool(name="ps", bufs=2, space="PSUM") as pspool,
        ):
            ones = cpool.tile([2, B], bf16)
            nc.vector.memset(ones[:], 1.0)
            tb = cpool.tile([2, PPC, D], bf16)
            nc.sync.dma_start(tb[:], bhl[:])
            ta = cpool.tile([128, PPC, NCHUNK, B], f32)
            for p in range(PPC):
                nc.sync.dma_start(ta[:, p], aT[p])

            slices = [(0, 512), (512, 768)]
            for j in range(NPAIR):
                p0, p1 = 2 * j, 2 * j + 1
                wt = wpool.tile([128, 2, NCHUNK, D], f32, tag="wt")
                nc.sync.dma_start(
                    wt[:], Wt[p0:p0 + 2].rearrange("p i c o -> i p c o")
                )
                pt = pspool.tile([2 * B, D], f32)
                for (o0, o1) in slices:
                    nc.tensor.matmul(
                        pt[:B, o0:o1], ones[:], tb[:, p0, o0:o1],
                        start=True, stop=False, tile_position=(0, 0),
                    )
                    nc.tensor.matmul(
                        pt[B:, o0:o1], ones[:], tb[:, p1, o0:o1],
                        start=True, stop=False, tile_position=(0, B),
                    )
                for c in range(NCHUNK):
                    for (o0, o1) in slices:
                        nc.tensor.matmul(
                            pt[:B, o0:o1], ta[:, p0, c, :], wt[:, 0, c, o0:o1],
                            start=False, stop=(c == NCHUNK - 1),
                            tile_position=(0, 0),
                        )
                        nc.tensor.matmul(
                            pt[B:, o0:o1], ta[:, p1, c, :], wt[:, 1, c, o0:o1],
                            start=False, stop=(c == NCHUNK - 1),
                            tile_position=(0, B),
                        )
                ob = opool.tile([2 * B, D], f32, tag="ob")
                nc.vector.tensor_copy(ob[:], pt[:])
                nc.sync.dma_start(outp[j], ob[:])

            # leftover patch (PPC is odd)
            p = PPC - 1
            wtl = wpool.tile([128, NCHUNK, D], f32, tag="wtl")
            nc.sync.dma_start(wtl[:], Wt[p])
            ptl = pspool.tile([B, D], f32, tag="ptl")
            for (o0, o1) in slices:
                nc.tensor.matmul(
                    ptl[:, o0:o1], ones[:], tb[:, p, o0:o1],
                    start=True, stop=False,
                )
            for c in range(NCHUNK):
                for (o0, o1) in slices:
                    nc.tensor.matmul(
                        ptl[:, o0:o1], ta[:, p, c, :], wtl[:, c, o0:o1],
                        start=False, stop=(c == NCHUNK - 1),
                    )
            obl = opool.tile([B, D], f32, tag="obl")
            nc.vector.tensor_copy(obl[:], ptl[:])
            nc.sync.dma_start(outl[:], obl[:])

    nc.finalize()
    return nc


def _build_d():
    """Mode C + per-patch W tiles (bufs=5, more outstanding DMAs), chunked aT
    preload, psum bufs=3."""
    nc = bacc.Bacc()
    NPAIR = PPC // 2
    aT = nc.declare_dram_parameter("aT", [PPC, 128, NCHUNK, B], f32, isOutput=False)
    Wt = nc.declare_dram_parameter("Wt", [PPC, 128, NCHUNK, D], f32, isOutput=False)
    bhl = nc.declare_dram_parameter("bhl", [2, PPC, D], bf16, isOutput=False)
    outp = nc.declare_dram_parameter("outp", [NPAIR, 2 * B, D], f32, isOutput=True)
    outl = nc.declare_dram_parameter("outl", [B, D], f32, isOutput=True)

    with tile.TileContext(nc) as tc:
        with (
            tc.tile_pool(name="const", bufs=1) as cpool,
            tc.tile_pool(name="w", bufs=5) as wpool,
            tc.tile_pool(name="o", bufs=3) as opool,
            tc.tile_pool(name="ps", bufs=3, space="PSUM") as pspool,
        ):
            ones = cpool.tile([2, B], bf16)
            nc.vector.memset(ones[:], 1.0)
            tb = cpool.tile([2, PPC, D], bf16)
            nc.sync.dma_start(tb[:], bhl[:])
            ta = cpool.tile([128, PPC, NCHUNK, B], f32)
            for p0 in range(0, PPC, 4):
                p1 = min(p0 + 4, PPC)
                nc.sync.dma_start(
                    ta[:, p0:p1], aT[p0:p1].rearrange("p i c b -> i p c b")
                )

            slices = [(0, 512), (512, 768)]

            def wtile(p):
                t = wpool.tile([128, NCHUNK, D], f32, tag="wt")
                nc.sync.dma_start(t[:], Wt[p])
                return t

            for j in range(NPAIR):
                p0, p1 = 2 * j, 2 * j + 1
                wt0 = wtile(p0)
                wt1 = wtile(p1)
                pt = pspool.tile([2 * B, D], f32)
                for (o0, o1) in slices:
                    nc.tensor.matmul(
                        pt[:B, o0:o1], ones[:], tb[:, p0, o0:o1],
                        start=True, stop=False, tile_position=(0, 0),
                    )
                    nc.tensor.matmul(
                        pt[B:, o0:o1], ones[:], tb[:, p1, o0:o1],
                        start=True, stop=False, tile_position=(0, B),
                    )
                for c in range(NCHUNK):
                    for (o0, o1) in slices:
                        nc.tensor.matmul(
                            pt[:B, o0:o1], ta[:, p0, c, :], wt0[:, c, o0:o1],
                            start=False, stop=(c == NCHUNK - 1),
                            tile_position=(0, 0),
                        )
                        nc.tensor.matmul(
                            pt[B:, o0:o1], ta[:, p1, c, :], wt1[:, c, o0:o1],
                            start=False, stop=(c == NCHUNK - 1),
                            tile_position=(0, B),
                        )
                ob = opool.tile([2 * B, D], f32, tag="ob")
                nc.vector.tensor_copy(ob[:], pt[:])
                nc.sync.dma_start(outp[j], ob[:])

            # leftover patch (PPC is odd)
            p = PPC - 1
            wtl = wtile(p)
            ptl = pspool.tile([B, D], f32, tag="pt")
            for (o0, o1) in slices:
                nc.tensor.matmul(
                    ptl[:, o0:o1], ones[:], tb[:, p, o0:o1],
                    start=True, stop=False,
                )
            for c in range(NCHUNK):
                for (o0, o1) in slices:
                    nc.tensor.matmul(
                        ptl[:, o0:o1], ta[:, p, c, :], wtl[:, c, o0:o1],
                        start=False, stop=(c == NCHUNK - 1),
                    )
            obl = opool.tile([B, D], f32, tag="obl")
            nc.vector.tensor_copy(obl[:], ptl[:])
            nc.sync.dma_start(outl[:], obl[:])

    nc.finalize()
    return nc


def _build_e():
    """Mode D + DMA spread across engines: W loads alternate between the SP
    and ACT HWDGE rings, aT/bias/output DMAs ride SWDGE (gpsimd). Leftover
    solo patch runs first so the kernel ends on a full pair."""
    nc = bacc.Bacc()
    NPAIR = PPC // 2
    aT = nc.declare_dram_parameter("aT", [PPC, 128, NCHUNK, B], f32, isOutput=False)
    Wt = nc.declare_dram_parameter("Wt", [PPC, 128, NCHUNK, D], f32, isOutput=False)
    bhl = nc.declare_dram_parameter("bhl", [2, PPC, D], bf16, isOutput=False)
    outp = nc.declare_dram_parameter("outp", [NPAIR, 2 * B, D], f32, isOutput=True)
    outl = nc.declare_dram_parameter("outl", [B, D], f32, isOutput=True)

    with tile.TileContext(nc) as tc:
        with (
            tc.tile_pool(name="const", bufs=1) as cpool,
            tc.tile_pool(name="w", bufs=5) as wpool,
            tc.tile_pool(name="o", bufs=3) as opool,
            tc.tile_pool(name="ps", bufs=3, space="PSUM") as pspool,
        ):
            ones = cpool.tile([2, B], bf16)
            nc.vector.memset(ones[:], 1.0)
            tb = cpool.tile([2, PPC, D], bf16)
            nc.gpsimd.dma_start(tb[:], bhl[:])
            ta = cpool.tile([128, PPC, NCHUNK, B], f32)
            for p0 in range(0, PPC, 4):
                p1 = min(p0 + 4, PPC)
                nc.gpsimd.dma_start(
                    ta[:, p0:p1], aT[p0:p1].rearrange("p i c b -> i p c b")
                )

            slices = [(0, 512), (512, 768)]
            _wcnt = [0]

            def wtile(p):
                t = wpool.tile([128, NCHUNK, D], f32, tag="wt")
                eng = nc.sync if _wcnt[0] % 2 == 0 else nc.scalar
                _wcnt[0] += 1
                eng.dma_start(t[:], Wt[p])
                return t

            # leftover solo patch first (PPC is odd)
            p = PPC - 1
            wtl = wtile(p)
            ptl = pspool.tile([B, D], f32, tag="pt")
            for (o0, o1) in slices:
                nc.tensor.matmul(
                    ptl[:, o0:o1], ones[:], tb[:, p, o0:o1],
                    start=True, stop=False,
                )
            for c in range(NCHUNK):
                for (o0, o1) in slices:
                    nc.tensor.matmul(
                        ptl[:, o0:o1], ta[:, p, c, :], wtl[:, c, o0:o1],
                        start=False, stop=(c == NCHUNK - 1),
                    )
            obl = opool.tile([B, D], f32, tag="obl")
            nc.vector.tensor_copy(obl[:], ptl[:])
            nc.gpsimd.dma_start(outl[:], obl[:])

            for j in range(NPAIR):
                p0, p1 = 2 * j, 2 * j + 1
                wt0 = wtile(p0)
                wt1 = wtile(p1)
                pt = pspool.tile([2 * B, D], f32, tag="pt")
                for (o0, o1) in slices:
                    nc.tensor.matmul(
                        pt[:B, o0:o1], ones[:], tb[:, p0, o0:o1],
                        start=True, stop=False, tile_position=(0, 0),
                    )
                    nc.tensor.matmul(
                        pt[B:, o0:o1], ones[:], tb[:, p1, o0:o1],
                        start=True, stop=False, tile_position=(0, B),
                    )
                for c in range(NCHUNK):
                    for (o0, o1) in slices:
                        nc.tensor.matmul(
                            pt[:B, o0:o1], ta[:, p0, c, :], wt0[:, c, o0:o1],
                            start=False, stop=(c == NCHUNK - 1),
                            tile_position=(0, 0),
                        )
                        nc.tensor.matmul(
                            pt[B:, o0:o1], ta[:, p1, c, :], wt1[:, c, o0:o1],
                            start=False, stop=(c == NCHUNK - 1),
                            tile_position=(0, B),
                        )
                ob = opool.tile([2 * B, D], f32, tag="ob")
                nc.vector.tensor_copy(ob[:], pt[:])
                nc.gpsimd.dma_start(outp[j], ob[:])

    nc.finalize()
    return nc


def _patchify(x):
    # [B, C, H, W] -> [B, 196, 768] in MAE ordering (n c h p w q -> n h w p q c)
    Bn, C, H, Wd = x.shape
    h = H // 16
    xr = x.reshape(Bn, C, h, 16, h, 16)
    xr = np.transpose(xr, (0, 2, 4, 3, 5, 1))
    return xr.reshape(Bn, h * h, 16 * 16 * C)


def kernel(x, W, b, _trace=False, _mode=None):
    global LAST_RESULTS
    mode = _mode or os.environ.get("KERNEL_MODE", "A")

    x = np.asarray(x, dtype=np.float32)
    W = np.asarray(W, dtype=np.float32)
    b = np.asarray(b, dtype=np.float32)

    patches = _patchify(x)                      # [64, 196, 768]

    in_maps = []
    for k in range(N_CORES):
        lo = k * PPC
        idx = np.arange(lo, lo + PPC)
        idx[idx >= NP] = 0                      # pad tail with patch 0
        psl = patches[:, idx, :]                # [64, 25, 768]
        wsl = W[idx]                            # [25, 768, 768]
        bsl = b[idx]                            # [25, 768]

        if mode in ("A2", "C", "D", "E"):
            aT = np.ascontiguousarray(
                psl.transpose(2, 1, 0)          # [768, 25, 64]
                .reshape(NCHUNK, 128, PPC, B)
                .transpose(2, 1, 0, 3)          # [25, 128, 6, 64]
            )
        else:
            aT = np.ascontiguousarray(
                psl.transpose(2, 1, 0)          # [768, 25, 64]
                .reshape(NCHUNK, 128, PPC, B)
                .transpose(1, 2, 0, 3)          # [128, 25, 6, 64]
            )
        Wt = np.ascontiguousarray(
            wsl.transpose(0, 2, 1)              # [25, 768(i), 768(o)]
            .reshape(PPC, NCHUNK, 128, D)
            .transpose(0, 2, 1, 3)              # [25, 128, 6, 768]
        )
        m = {"aT": aT, "Wt": Wt}
        if mode == "A":
            hi = bsl.astype(ml_dtypes.bfloat16)
            lo_ = (bsl - hi.astype(np.float32)).astype(ml_dtypes.bfloat16)
            m["bhl"] = np.ascontiguousarray(np.stack([hi, lo_], axis=0))
        elif mode in ("A2", "C", "D", "E"):
            hi = bsl.astype(ml_dtypes.bfloat16)
            lo_ = (bsl - hi.astype(np.float32)).astype(ml_dtypes.bfloat16)
            m["bhl"] = np.ascontiguousarray(np.stack([hi, lo_], axis=0))
        else:
            m["bpp"] = np.ascontiguousarray(
                bsl.reshape(PPC, NCHUNK, 128).transpose(2, 0, 1)
            )
        in_maps.append(m)

    key = mode
    if key not in _NC_CACHE:
        _NC_CACHE[key] = _build(mode)
    nc = _NC_CACHE[key]

    res = run_bass_kernel_spmd(nc, in_maps, list(range(N_CORES)), trace=_trace)
    LAST_RESULTS = res

    if mode in ("C", "D", "E"):
        parts = np.concatenate(
            [
                np.concatenate(
                    [
                        res.results[k]["outp"].reshape(PPC - 1, B, D),
                        res.results[k]["outl"][None],
                    ],
                    axis=0,
                )[None]
                for k in range(N_CORES)
            ]
        )                                       # [8, 25, 64, 768]
        full = parts.transpose(2, 0, 1, 3).reshape(B, N_CORES * PPC, D)
    else:
        parts = np.stack([res.results[k]["out"] for k in range(N_CORES)])
        if mode in ("A", "A2"):
            # parts [8, 25, 64, 768] -> [64, 200, 768]
            full = parts.transpose(2, 0, 1, 3).reshape(B, N_CORES * PPC, D)
        else:
            # parts [8, 25, 128(o_in), 6(oc), 64(b)] -> [64, 200, 768]
            full = parts.transpose(4, 0, 1, 3, 2).reshape(B, N_CORES * PPC, D)
    return np.ascontiguousarray(full[:, :NP, :])


# revision 18
# speedup vs baseline: 1.1227x; 1.1227x over previous
"""Trainium2 kernel for nn_NonsharedPatchEmbed: 196 independent Linear(768->768)
applied per patch of a patchified [64, 3, 224, 224] image batch.

  out[b, p, o] = sum_i patches[b, p, i] * W[p, o, i] + b[p, o]

Strategy: shard the 196-patch axis across the 8 NeuronCores (25 patches per
core, padded to 200). Per patch this is a [64, 768] x [768, 768]^T GEMM with
the full batch as the stationary operand; the per-patch W (the dominant
traffic, 462 MB total) streams through the tensor engine exactly once.

Host-side work is layout only: patchify x, pre-transpose W to W^T, and split
the bias into a bf16 hi+lo pair (added exactly via a K=2 ones-matmul into the
same PSUM accumulation group).

Layouts per core (mode A, W moving):
  aT  [128, 25, 6, 64]  f32   aT[i, p, c, b] = patches[b, 25k+p, 128c+i]
  Wt  [25, 128, 6, 768] f32   Wt[p, i, c, o] = W[25k+p, o, 128c+i]
  bhl [2, 25, 768]      bf16  bias hi/lo split
  out [25, 64, 768]     f32

Mode B (W stationary): same inputs plus bias_pp [128, 25, 6] f32; per
(patch, o_chunk) accumulates psum [128, 64] over 6 i-chunks with W^T blocks
as lhsT; bias added per-partition during the PSUM->SBUF copy.
"""

import os
import numpy as np
import ml_dtypes

import concourse.bass as bass
import concourse.tile as tile
import concourse.mybir as mybir
from concourse import bacc
from concourse.bass_utils import run_bass_kernel_spmd

f32 = mybir.dt.float32
bf16 = mybir.dt.bfloat16

N_CORES = 8
B = 64
D = 768
NP = 196
PPC = 25          # patches per core (8*25 = 200, last 4 padded)
NCHUNK = 6        # 768 / 128

LAST_RESULTS = None  # BassKernelResults of the most recent run (for test.py)

_NC_CACHE = {}


def _build(mode):
    if mode == "A2":
        return _build_a2()
    if mode == "C":
        return _build_c()
    if mode == "D":
        return _build_d()
    if mode == "E":
        return _build_e()
    if mode == "D2":
        return _build_d2()
    nc = bacc.Bacc()
    aT = nc.declare_dram_parameter("aT", [128, PPC, NCHUNK, B], f32, isOutput=False)
    Wt = nc.declare_dram_parameter("Wt", [PPC, 128, NCHUNK, D], f32, isOutput=False)
    if mode == "A":
        bhl = nc.declare_dram_parameter("bhl", [2, PPC, D], bf16, isOutput=False)
        out = nc.declare_dram_parameter("out", [PPC, B, D], f32, isOutput=True)
    else:
        bpp = nc.declare_dram_parameter("bpp", [128, PPC, NCHUNK], f32, isOutput=False)
        out = nc.declare_dram_parameter("out", [PPC, 128, NCHUNK, B], f32, isOutput=True)

    with tile.TileContext(nc) as tc:
        with (
            tc.tile_pool(name="const", bufs=1) as cpool,
            tc.tile_pool(name="w", bufs=3) as wpool,
            tc.tile_pool(name="o", bufs=3) as opool,
            tc.tile_pool(name="ps", bufs=4, space="PSUM") as pspool,
        ):
            ta = cpool.tile([128, PPC, NCHUNK, B], f32)
            nc.sync.dma_start(ta[:], aT[:])
            if mode == "A":
                ones = cpool.tile([2, B], bf16)
                nc.vector.memset(ones[:], 1.0)
                tb = cpool.tile([2, PPC, D], bf16)
                nc.sync.dma_start(tb[:], bhl[:])
            else:
                tbias = cpool.tile([128, PPC, NCHUNK], f32)
                nc.sync.dma_start(tbias[:], bpp[:])

            for p in range(PPC):
                wt = wpool.tile([128, NCHUNK, D], f32)
                nc.sync.dma_start(wt[:], Wt[p])

                if mode == "A":
                    pt = pspool.tile([B, D], f32)
                    slices = [(0, 512), (512, 768)]
                    for (o0, o1) in slices:
                        nc.tensor.matmul(
                            pt[:, o0:o1], ones[:], tb[:, p, o0:o1],
                            start=True, stop=False,
                        )
                    for c in range(NCHUNK):
                        for (o0, o1) in slices:
                            nc.tensor.matmul(
                                pt[:, o0:o1], ta[:, p, c, :], wt[:, c, o0:o1],
                                start=False, stop=(c == NCHUNK - 1),
                            )
                    ob = opool.tile([B, D], f32)
                    nc.vector.tensor_copy(ob[:], pt[:])
                    nc.sync.dma_start(out[p], ob[:])
                else:
                    ob = opool.tile([128, NCHUNK, B], f32)
                    for oc in range(NCHUNK):
                        pt = pspool.tile([128, B], f32)
                        for c in range(NCHUNK):
                            nc.tensor.matmul(
                                pt[:], wt[:, c, oc * 128:(oc + 1) * 128],
                                ta[:, p, c, :],
                                start=(c == 0), stop=(c == NCHUNK - 1),
                            )
                        nc.vector.tensor_scalar_add(
                            ob[:, oc, :], pt[:], tbias[:, p, oc:oc + 1]
                        )
                    nc.sync.dma_start(out[p], ob[:])

    nc.finalize()
    return nc


def _build_a2():
    """Mode A with: per-patch aT loads (fast start) and bias added on DVE via
    partition-broadcast during the PSUM->SBUF copy (no PE bias matmuls)."""
    nc = bacc.Bacc()
    aT = nc.declare_dram_parameter("aT", [PPC, 128, NCHUNK, B], f32, isOutput=False)
    Wt = nc.declare_dram_parameter("Wt", [PPC, 128, NCHUNK, D], f32, isOutput=False)
    bhl = nc.declare_dram_parameter("bhl", [2, PPC, D], bf16, isOutput=False)
    out = nc.declare_dram_parameter("out", [PPC, B, D], f32, isOutput=True)

    with tile.TileContext(nc) as tc:
        with (
            tc.tile_pool(name="const", bufs=1) as cpool,
            tc.tile_pool(name="w", bufs=3) as wpool,
            tc.tile_pool(name="o", bufs=3) as opool,
            tc.tile_pool(name="ps", bufs=4, space="PSUM") as pspool,
        ):
            ones = cpool.tile([2, B], bf16)
            nc.vector.memset(ones[:], 1.0)
            tb = cpool.tile([2, PPC, D], bf16)
            nc.sync.dma_start(tb[:], bhl[:])
            ta = cpool.tile([128, PPC, NCHUNK, B], f32)
            for p in range(PPC):
                nc.sync.dma_start(ta[:, p], aT[p])

            slices = [(0, 512), (512, 768)]
            for p in range(PPC):
                wt = wpool.tile([128, NCHUNK, D], f32)
                nc.sync.dma_start(wt[:], Wt[p])
                pt = pspool.tile([B, D], f32)
                for (o0, o1) in slices:
                    nc.tensor.matmul(
                        pt[:, o0:o1], ones[:], tb[:, p, o0:o1],
                        start=True, stop=False,
                    )
                for c in range(NCHUNK):
                    for (o0, o1) in slices:
                        nc.tensor.matmul(
                            pt[:, o0:o1], ta[:, p, c, :], wt[:, c, o0:o1],
                            start=False, stop=(c == NCHUNK - 1),
                        )
                ob = opool.tile([B, D], f32)
                nc.vector.tensor_copy(ob[:], pt[:])
                nc.sync.dma_start(out[p], ob[:])

    nc.finalize()
    return nc


def _build_c():
    """Col-tiled patch pairs: two patches share the 128x128 array (cols 0-63 /
    64-127), each streaming its own W. Full-partition PSUM + output DMAs."""
    nc = bacc.Bacc()
    NPAIR = PPC // 2           # 12 pairs + 1 leftover patch
    aT = nc.declare_dram_parameter("aT", [PPC, 128, NCHUNK, B], f32, isOutput=False)
    Wt = nc.declare_dram_parameter("Wt", [PPC, 128, NCHUNK, D], f32, isOutput=False)
    bhl = nc.declare_dram_parameter("bhl", [2, PPC, D], bf16, isOutput=False)
    outp = nc.declare_dram_parameter("outp", [NPAIR, 2 * B, D], f32, isOutput=True)
    outl = nc.declare_dram_parameter("outl", [B, D], f32, isOutput=True)

    with tile.TileContext(nc) as tc:
        with (
            tc.tile_pool(name="const", bufs=1) as cpool,
            tc.tile_pool(name="w", bufs=2) as wpool,
            tc.tile_pool(name="o", bufs=3) as opool,
            tc.tile_pool(name="ps", bufs=2, space="PSUM") as pspool,
        ):
            ones = cpool.tile([2, B], bf16)
            nc.vector.memset(ones[:], 1.0)
            tb = cpool.tile([2, PPC, D], bf16)
            nc.sync.dma_start(tb[:], bhl[:])
            ta = cpool.tile([128, PPC, NCHUNK, B], f32)
            for p in range(PPC):
                nc.sync.dma_start(ta[:, p], aT[p])

            slices = [(0, 512), (512, 768)]
            for j in range(NPAIR):
                p0, p1 = 2 * j, 2 * j + 1
                wt = wpool.tile([128, 2, NCHUNK, D], f32, tag="wt")
                nc.sync.dma_start(
                    wt[:], Wt[p0:p0 + 2].rearrange("p i c o -> i p c o")
                )
                pt = pspool.tile([2 * B, D], f32)
                for (o0, o1) in slices:
                    nc.tensor.matmul(
                        pt[:B, o0:o1], ones[:], tb[:, p0, o0:o1],
                        start=True, stop=False, tile_position=(0, 0),
                    )
                    nc.tensor.matmul(
                        pt[B:, o0:o1], ones[:], tb[:, p1, o0:o1],
                        start=True, stop=False, tile_position=(0, B),
                    )
                for c in range(NCHUNK):
                    for (o0, o1) in slices:
                        nc.tensor.matmul(
                            pt[:B, o0:o1], ta[:, p0, c, :], wt[:, 0, c, o0:o1],
                            start=False, stop=(c == NCHUNK - 1),
                            tile_position=(0, 0),
                        )
                        nc.tensor.matmul(
                            pt[B:, o0:o1], ta[:, p1, c, :], wt[:, 1, c, o0:o1],
                            start=False, stop=(c == NCHUNK - 1),
                            tile_position=(0, B),
                        )
                ob = opool.tile([2 * B, D], f32, tag="ob")
                nc.vector.tensor_copy(ob[:], pt[:])
                nc.sync.dma_start(outp[j], ob[:])

            # leftover patch (PPC is odd)
            p = PPC - 1
            wtl = wpool.tile([128, NCHUNK, D], f32, tag="wtl")
            nc.sync.dma_start(wtl[:], Wt[p])
            ptl = pspool.tile([B, D], f32, tag="ptl")
            for (o0, o1) in slices:
                nc.tensor.matmul(
                    ptl[:, o0:o1], ones[:], tb[:, p, o0:o1],
                    start=True, stop=False,
                )
            for c in range(NCHUNK):
                for (o0, o1) in slices:
                    nc.tensor.matmul(
                        ptl[:, o0:o1], ta[:, p, c, :], wtl[:, c, o0:o1],
                        start=False, stop=(c == NCHUNK - 1),
                    )
            obl = opool.tile([B, D], f32, tag="obl")
            nc.vector.tensor_copy(obl[:], ptl[:])
            nc.sync.dma_start(outl[:], obl[:])

    nc.finalize()
    return nc


def _build_d():
    """Mode C + per-patch W tiles (bufs=5, more outstanding DMAs), chunked aT
    preload, psum bufs=3."""
    nc = bacc.Bacc()
    NPAIR = PPC // 2
    aT = nc.declare_dram_parameter("aT", [PPC, 128, NCHUNK, B], f32, isOutput=False)
    Wt = nc.declare_dram_parameter("Wt", [PPC, 128, NCHUNK, D], f32, isOutput=False)
    bhl = nc.declare_dram_parameter("bhl", [2, PPC, D], bf16, isOutput=False)
    outp = nc.declare_dram_parameter("outp", [NPAIR, 2 * B, D], f32, isOutput=True)
    outl = nc.declare_dram_parameter("outl", [B, D], f32, isOutput=True)

    with tile.TileContext(nc) as tc:
        with (
            tc.tile_pool(name="const", bufs=1) as cpool,
            tc.tile_pool(name="w", bufs=5) as wpool,
            tc.tile_pool(name="o", bufs=3) as opool,
            tc.tile_pool(name="ps", bufs=3, space="PSUM") as pspool,
        ):
            ones = cpool.tile([2, B], bf16)
            nc.vector.memset(ones[:], 1.0)
            tb = cpool.tile([2, PPC, D], bf16)
            nc.sync.dma_start(tb[:], bhl[:])
            ta = cpool.tile([128, PPC, NCHUNK, B], f32)
            for p0 in range(0, PPC, 4):
                p1 = min(p0 + 4, PPC)
                nc.sync.dma_start(
                    ta[:, p0:p1], aT[p0:p1].rearrange("p i c b -> i p c b")
                )

            slices = [(0, 512), (512, 768)]

            def wtile(p):
                t = wpool.tile([128, NCHUNK, D], f32, tag="wt")
                nc.sync.dma_start(t[:], Wt[p])
                return t

            for j in range(NPAIR):
                p0, p1 = 2 * j, 2 * j + 1
                wt0 = wtile(p0)
                wt1 = wtile(p1)
                pt = pspool.tile([2 * B, D], f32)
                for (o0, o1) in slices:
                    nc.tensor.matmul(
                        pt[:B, o0:o1], ones[:], tb[:, p0, o0:o1],
                        start=True, stop=False, tile_position=(0, 0),
                    )
                    nc.tensor.matmul(
                        pt[B:, o0:o1], ones[:], tb[:, p1, o0:o1],
                        start=True, stop=False, tile_position=(0, B),
                    )
                for c in range(NCHUNK):
                    for (o0, o1) in slices:
                        nc.tensor.matmul(
                            pt[:B, o0:o1], ta[:, p0, c, :], wt0[:, c, o0:o1],
                            start=False, stop=(c == NCHUNK - 1),
                            tile_position=(0, 0),
                        )
                        nc.tensor.matmul(
                            pt[B:, o0:o1], ta[:, p1, c, :], wt1[:, c, o0:o1],
                            start=False, stop=(c == NCHUNK - 1),
                            tile_position=(0, B),
                        )
                ob = opool.tile([2 * B, D], f32, tag="ob")
                nc.vector.tensor_copy(ob[:], pt[:])
                nc.sync.dma_start(outp[j], ob[:])

            # leftover patch (PPC is odd)
            p = PPC - 1
            wtl = wtile(p)
            ptl = pspool.tile([B, D], f32, tag="pt")
            for (o0, o1) in slices:
                nc.tensor.matmul(
                    ptl[:, o0:o1], ones[:], tb[:, p, o0:o1],
                    start=True, stop=False,
                )
            for c in range(NCHUNK):
                for (o0, o1) in slices:
                    nc.tensor.matmul(
                        ptl[:, o0:o1], ta[:, p, c, :], wtl[:, c, o0:o1],
                        start=False, stop=(c == NCHUNK - 1),
                    )
            obl = opool.tile([B, D], f32, tag="obl")
            nc.vector.tensor_copy(obl[:], ptl[:])
            nc.sync.dma_start(outl[:], obl[:])

    nc.finalize()
    return nc


def _build_e():
    """Mode D + DMA spread across engines: W loads alternate between the SP
    and ACT HWDGE rings, aT/bias/output DMAs ride SWDGE (gpsimd). Leftover
    solo patch runs first so the kernel ends on a full pair."""
    nc = bacc.Bacc()
    NPAIR = PPC // 2
    aT = nc.declare_dram_parameter("aT", [PPC, 128, NCHUNK, B], f32, isOutput=False)
    Wt = nc.declare_dram_parameter("Wt", [PPC, 128, NCHUNK, D], f32, isOutput=False)
    bhl = nc.declare_dram_parameter("bhl", [2, PPC, D], bf16, isOutput=False)
    outp = nc.declare_dram_parameter("outp", [NPAIR, 2 * B, D], f32, isOutput=True)
    outl = nc.declare_dram_parameter("outl", [B, D], f32, isOutput=True)

    with tile.TileContext(nc) as tc:
        with (
            tc.tile_pool(name="const", bufs=1) as cpool,
            tc.tile_pool(name="w", bufs=5) as wpool,
            tc.tile_pool(name="o", bufs=3) as opool,
            tc.tile_pool(name="ps", bufs=3, space="PSUM") as pspool,
        ):
            ones = cpool.tile([2, B], bf16)
            nc.vector.memset(ones[:], 1.0)
            tb = cpool.tile([2, PPC, D], bf16)
            nc.gpsimd.dma_start(tb[:], bhl[:])
            ta = cpool.tile([128, PPC, NCHUNK, B], f32)
            for p0 in range(0, PPC, 4):
                p1 = min(p0 + 4, PPC)
                nc.gpsimd.dma_start(
                    ta[:, p0:p1], aT[p0:p1].rearrange("p i c b -> i p c b")
                )

            slices = [(0, 512), (512, 768)]
            _wcnt = [0]

            def wtile(p):
                t = wpool.tile([128, NCHUNK, D], f32, tag="wt")
                eng = nc.sync if _wcnt[0] % 2 == 0 else nc.scalar
                _wcnt[0] += 1
                eng.dma_start(t[:], Wt[p])
                return t

            # leftover solo patch first (PPC is odd)
            p = PPC - 1
            wtl = wtile(p)
            ptl = pspool.tile([B, D], f32, tag="pt")
            for (o0, o1) in slices:
                nc.tensor.matmul(
                    ptl[:, o0:o1], ones[:], tb[:, p, o0:o1],
                    start=True, stop=False,
                )
            for c in range(NCHUNK):
                for (o0, o1) in slices:
                    nc.tensor.matmul(
                        ptl[:, o0:o1], ta[:, p, c, :], wtl[:, c, o0:o1],
                        start=False, stop=(c == NCHUNK - 1),
                    )
            obl = opool.tile([B, D], f32, tag="obl")
            nc.vector.tensor_copy(obl[:], ptl[:])
            nc.gpsimd.dma_start(outl[:], obl[:])

            for j in range(NPAIR):
                p0, p1 = 2 * j, 2 * j + 1
                wt0 = wtile(p0)
                wt1 = wtile(p1)
                pt = pspool.tile([2 * B, D], f32, tag="pt")
                for (o0, o1) in slices:
                    nc.tensor.matmul(
                        pt[:B, o0:o1], ones[:], tb[:, p0, o0:o1],
                        start=True, stop=False, tile_position=(0, 0),
                    )
                    nc.tensor.matmul(
                        pt[B:, o0:o1], ones[:], tb[:, p1, o0:o1],
                        start=True, stop=False, tile_position=(0, B),
                    )
                for c in range(NCHUNK):
                    for (o0, o1) in slices:
                        nc.tensor.matmul(
                            pt[:B, o0:o1], ta[:, p0, c, :], wt0[:, c, o0:o1],
                            start=False, stop=(c == NCHUNK - 1),
                            tile_position=(0, 0),
                        )
                        nc.tensor.matmul(
                            pt[B:, o0:o1], ta[:, p1, c, :], wt1[:, c, o0:o1],
                            start=False, stop=(c == NCHUNK - 1),
                            tile_position=(0, B),
                        )
                ob = opool.tile([2 * B, D], f32, tag="ob")
                nc.vector.tensor_copy(ob[:], pt[:])
                nc.gpsimd.dma_start(outp[j], ob[:])

    nc.finalize()
    return nc


def _build_d2():
    """Mode D + solo patch first; W stream pure on the SP HWDGE ring; aT/bias/
    output DMAs on the ACT HWDGE ring; deeper W prefetch."""
    nc = bacc.Bacc()
    NPAIR = PPC // 2
    aT = nc.declare_dram_parameter("aT", [PPC, 128, NCHUNK, B], f32, isOutput=False)
    Wt = nc.declare_dram_parameter("Wt", [PPC, 128, NCHUNK, D], f32, isOutput=False)
    bhl = nc.declare_dram_parameter("bhl", [2, PPC, D], bf16, isOutput=False)
    outp = nc.declare_dram_parameter("outp", [NPAIR, 2 * B, D], f32, isOutput=True)
    outl = nc.declare_dram_parameter("outl", [B, D], f32, isOutput=True)

    with tile.TileContext(nc) as tc:
        with (
            tc.tile_pool(name="const", bufs=1) as cpool,
            tc.tile_pool(name="w", bufs=6) as wpool,
            tc.tile_pool(name="o", bufs=3) as opool,
            tc.tile_pool(name="ps", bufs=3, space="PSUM") as pspool,
        ):
            ones = cpool.tile([2, B], bf16)
            nc.vector.memset(ones[:], 1.0)
            tb = cpool.tile([2, PPC, D], bf16)
            nc.scalar.dma_start(tb[:], bhl[:])
            ta = cpool.tile([128, PPC, NCHUNK, B], f32)
            for p0 in range(0, PPC, 4):
                p1 = min(p0 + 4, PPC)
                nc.scalar.dma_start(
                    ta[:, p0:p1], aT[p0:p1].rearrange("p i c b -> i p c b")
                )

            slices = [(0, 512), (512, 768)]
            _wcnt = [0]

            def wtile(p):
                t = wpool.tile([128, NCHUNK, D], f32, tag="wt")
                _wcnt[0] += 1
                nc.sync.dma_start(t[:], Wt[p])
                return t

            # leftover solo patch first (PPC is odd)
            p = PPC - 1
            wtl = wtile(p)
            ptl = pspool.tile([B, D], f32, tag="pt")
            for (o0, o1) in slices:
                nc.tensor.matmul(
                    ptl[:, o0:o1], ones[:], tb[:, p, o0:o1],
                    start=True, stop=False,
                )
            for c in range(NCHUNK):
                for (o0, o1) in slices:
                    nc.tensor.matmul(
                        ptl[:, o0:o1], ta[:, p, c, :], wtl[:, c, o0:o1],
                        start=False, stop=(c == NCHUNK - 1),
                    )
            obl = opool.tile([B, D], f32, tag="obl")
            nc.vector.tensor_copy(obl[:], ptl[:])
            nc.scalar.dma_start(outl[:], obl[:])

            for j in range(NPAIR):
                p0, p1 = 2 * j, 2 * j + 1
                wt0 = wtile(p0)
                wt1 = wtile(p1)
                pt = pspool.tile([2 * B, D], f32, tag="pt")
                for (o0, o1) in slices:
                    nc.tensor.matmul(
                        pt[:B, o0:o1], ones[:], tb[:, p0, o0:o1],
                        start=True, stop=False, tile_position=(0, 0),
                    )
                    nc.tensor.matmul(
                        pt[B:, o0:o1], ones[:], tb[:, p1, o0:o1],
                        start=True, stop=False, tile_position=(0, B),
                    )
                for c in range(NCHUNK):
                    for (o0, o1) in slices:
                        nc.tensor.matmul(
                            pt[:B, o0:o1], ta[:, p0, c, :], wt0[:, c, o0:o1],
                            start=False, stop=(c == NCHUNK - 1),
                            tile_position=(0, 0),
                        )
                        nc.tensor.matmul(
                            pt[B:, o0:o1], ta[:, p1, c, :], wt1[:, c, o0:o1],
                            start=False, stop=(c == NCHUNK - 1),
                            tile_position=(0, B),
                        )
                ob = opool.tile([2 * B, D], f32, tag="ob")
                nc.vector.tensor_copy(ob[:], pt[:])
                nc.scalar.dma_start(outp[j], ob[:])

    nc.finalize()
    return nc


def _patchify(x):
    # [B, C, H, W] -> [B, 196, 768] in MAE ordering (n c h p w q -> n h w p q c)
    Bn, C, H, Wd = x.shape
    h = H // 16
    xr = x.reshape(Bn, C, h, 16, h, 16)
    xr = np.transpose(xr, (0, 2, 4, 3, 5, 1))
    return xr.reshape(Bn, h * h, 16 * 16 * C)


def kernel(x, W, b, _trace=False, _mode=None):
    global LAST_RESULTS
    mode = _mode or os.environ.get("KERNEL_MODE", "A")

    x = np.asarray(x, dtype=np.float32)
    W = np.asarray(W, dtype=np.float32)
    b = np.asarray(b, dtype=np.float32)

    patches = _patchify(x)                      # [64, 196, 768]

    in_maps = []
    for k in range(N_CORES):
        lo = k * PPC
        idx = np.arange(lo, lo + PPC)
        idx[idx >= NP] = 0                      # pad tail with patch 0
        psl = patches[:, idx, :]                # [64, 25, 768]
        wsl = W[idx]                            # [25, 768, 768]
        bsl = b[idx]                            # [25, 768]

        if mode in ("A2", "C", "D", "E", "D2"):
            aT = np.ascontiguousarray(
                psl.transpose(2, 1, 0)          # [768, 25, 64]
                .reshape(NCHUNK, 128, PPC, B)
                .transpose(2, 1, 0, 3)          # [25, 128, 6, 64]
            )
        else:
            aT = np.ascontiguousarray(
                psl.transpose(2, 1, 0)          # [768, 25, 64]
                .reshape(NCHUNK, 128, PPC, B)
                .transpose(1, 2, 0, 3)          # [128, 25, 6, 64]
            )
        Wt = np.ascontiguousarray(
            wsl.transpose(0, 2, 1)              # [25, 768(i), 768(o)]
            .reshape(PPC, NCHUNK, 128, D)
            .transpose(0, 2, 1, 3)              # [25, 128, 6, 768]
        )
        m = {"aT": aT, "Wt": Wt}
        if mode == "A":
            hi = bsl.astype(ml_dtypes.bfloat16)
            lo_ = (bsl - hi.astype(np.float32)).astype(ml_dtypes.bfloat16)
            m["bhl"] = np.ascontiguousarray(np.stack([hi, lo_], axis=0))
        elif mode in ("A2", "C", "D", "E", "D2"):
            hi = bsl.astype(ml_dtypes.bfloat16)
            lo_ = (bsl - hi.astype(np.float32)).astype(ml_dtypes.bfloat16)
            m["bhl"] = np.ascontiguousarray(np.stack([hi, lo_], axis=0))
        else:
            m["bpp"] = np.ascontiguousarray(
                bsl.reshape(PPC, NCHUNK, 128).transpose(2, 0, 1)
            )
        in_maps.append(m)

    key = mode
    if key not in _NC_CACHE:
        _NC_CACHE[key] = _build(mode)
    nc = _NC_CACHE[key]

    res = run_bass_kernel_spmd(nc, in_maps, list(range(N_CORES)), trace=_trace)
    LAST_RESULTS = res

    if mode in ("C", "D", "E", "D2"):
        parts = np.concatenate(
            [
                np.concatenate(
                    [
                        res.results[k]["outp"].reshape(PPC - 1, B, D),
                        res.results[k]["outl"][None],
                    ],
                    axis=0,
                )[None]
                for k in range(N_CORES)
            ]
        )                                       # [8, 25, 64, 768]
        full = parts.transpose(2, 0, 1, 3).reshape(B, N_CORES * PPC, D)
    else:
        parts = np.stack([res.results[k]["out"] for k in range(N_CORES)])
        if mode in ("A", "A2"):
            # parts [8, 25, 64, 768] -> [64, 200, 768]
            full = parts.transpose(2, 0, 1, 3).reshape(B, N_CORES * PPC, D)
        else:
            # parts [8, 25, 128(o_in), 6(oc), 64(b)] -> [64, 200, 768]
            full = parts.transpose(4, 0, 1, 3, 2).reshape(B, N_CORES * PPC, D)
    return np.ascontiguousarray(full[:, :NP, :])


# revision 19
# speedup vs baseline: 1.1465x; 1.0213x over previous
"""Trainium2 kernel for nn_NonsharedPatchEmbed: 196 independent Linear(768->768)
applied per patch of a patchified [64, 3, 224, 224] image batch.

  out[b, p, o] = sum_i patches[b, p, i] * W[p, o, i] + b[p, o]

Strategy: shard the 196-patch axis across the 8 NeuronCores (25 patches per
core, padded to 200). Per patch this is a [64, 768] x [768, 768]^T GEMM with
the full batch as the stationary operand; the per-patch W (the dominant
traffic, 462 MB total) streams through the tensor engine exactly once.

Host-side work is layout only: patchify x, pre-transpose W to W^T, and split
the bias into a bf16 hi+lo pair (added exactly via a K=2 ones-matmul into the
same PSUM accumulation group).

Layouts per core (mode A, W moving):
  aT  [128, 25, 6, 64]  f32   aT[i, p, c, b] = patches[b, 25k+p, 128c+i]
  Wt  [25, 128, 6, 768] f32   Wt[p, i, c, o] = W[25k+p, o, 128c+i]
  bhl [2, 25, 768]      bf16  bias hi/lo split
  out [25, 64, 768]     f32

Mode B (W stationary): same inputs plus bias_pp [128, 25, 6] f32; per
(patch, o_chunk) accumulates psum [128, 64] over 6 i-chunks with W^T blocks
as lhsT; bias added per-partition during the PSUM->SBUF copy.
"""

import os
import numpy as np
import ml_dtypes

import concourse.bass as bass
import concourse.tile as tile
import concourse.mybir as mybir
from concourse import bacc
from concourse.bass_utils import run_bass_kernel_spmd

f32 = mybir.dt.float32
bf16 = mybir.dt.bfloat16

N_CORES = 8
B = 64
D = 768
NP = 196
PPC = 25          # patches per core (8*25 = 200, last 4 padded)
NCHUNK = 6        # 768 / 128

LAST_RESULTS = None  # BassKernelResults of the most recent run (for test.py)

_NC_CACHE = {}


def _build(mode):
    if mode == "A2":
        return _build_a2()
    if mode == "C":
        return _build_c()
    if mode == "D":
        return _build_d()
    if mode == "E":
        return _build_e()
    if mode == "D2":
        return _build_d2()
    if mode == "D3":
        return _build_d3()
    nc = bacc.Bacc()
    aT = nc.declare_dram_parameter("aT", [128, PPC, NCHUNK, B], f32, isOutput=False)
    Wt = nc.declare_dram_parameter("Wt", [PPC, 128, NCHUNK, D], f32, isOutput=False)
    if mode == "A":
        bhl = nc.declare_dram_parameter("bhl", [2, PPC, D], bf16, isOutput=False)
        out = nc.declare_dram_parameter("out", [PPC, B, D], f32, isOutput=True)
    else:
        bpp = nc.declare_dram_parameter("bpp", [128, PPC, NCHUNK], f32, isOutput=False)
        out = nc.declare_dram_parameter("out", [PPC, 128, NCHUNK, B], f32, isOutput=True)

    with tile.TileContext(nc) as tc:
        with (
            tc.tile_pool(name="const", bufs=1) as cpool,
            tc.tile_pool(name="w", bufs=3) as wpool,
            tc.tile_pool(name="o", bufs=3) as opool,
            tc.tile_pool(name="ps", bufs=4, space="PSUM") as pspool,
        ):
            ta = cpool.tile([128, PPC, NCHUNK, B], f32)
            nc.sync.dma_start(ta[:], aT[:])
            if mode == "A":
                ones = cpool.tile([2, B], bf16)
                nc.vector.memset(ones[:], 1.0)
                tb = cpool.tile([2, PPC, D], bf16)
                nc.sync.dma_start(tb[:], bhl[:])
            else:
                tbias = cpool.tile([128, PPC, NCHUNK], f32)
                nc.sync.dma_start(tbias[:], bpp[:])

            for p in range(PPC):
                wt = wpool.tile([128, NCHUNK, D], f32)
                nc.sync.dma_start(wt[:], Wt[p])

                if mode == "A":
                    pt = pspool.tile([B, D], f32)
                    slices = [(0, 512), (512, 768)]
                    for (o0, o1) in slices:
                        nc.tensor.matmul(
                            pt[:, o0:o1], ones[:], tb[:, p, o0:o1],
                            start=True, stop=False,
                        )
                    for c in range(NCHUNK):
                        for (o0, o1) in slices:
                            nc.tensor.matmul(
                                pt[:, o0:o1], ta[:, p, c, :], wt[:, c, o0:o1],
                                start=False, stop=(c == NCHUNK - 1),
                            )
                    ob = opool.tile([B, D], f32)
                    nc.vector.tensor_copy(ob[:], pt[:])
                    nc.sync.dma_start(out[p], ob[:])
                else:
                    ob = opool.tile([128, NCHUNK, B], f32)
                    for oc in range(NCHUNK):
                        pt = pspool.tile([128, B], f32)
                        for c in range(NCHUNK):
                            nc.tensor.matmul(
                                pt[:], wt[:, c, oc * 128:(oc + 1) * 128],
                                ta[:, p, c, :],
                                start=(c == 0), stop=(c == NCHUNK - 1),
                            )
                        nc.vector.tensor_scalar_add(
                            ob[:, oc, :], pt[:], tbias[:, p, oc:oc + 1]
                        )
                    nc.sync.dma_start(out[p], ob[:])

    nc.finalize()
    return nc


def _build_a2():
    """Mode A with: per-patch aT loads (fast start) and bias added on DVE via
    partition-broadcast during the PSUM->SBUF copy (no PE bias matmuls)."""
    nc = bacc.Bacc()
    aT = nc.declare_dram_parameter("aT", [PPC, 128, NCHUNK, B], f32, isOutput=False)
    Wt = nc.declare_dram_parameter("Wt", [PPC, 128, NCHUNK, D], f32, isOutput=False)
    bhl = nc.declare_dram_parameter("bhl", [2, PPC, D], bf16, isOutput=False)
    out = nc.declare_dram_parameter("out", [PPC, B, D], f32, isOutput=True)

    with tile.TileContext(nc) as tc:
        with (
            tc.tile_pool(name="const", bufs=1) as cpool,
            tc.tile_pool(name="w", bufs=3) as wpool,
            tc.tile_pool(name="o", bufs=3) as opool,
            tc.tile_pool(name="ps", bufs=4, space="PSUM") as pspool,
        ):
            ones = cpool.tile([2, B], bf16)
            nc.vector.memset(ones[:], 1.0)
            tb = cpool.tile([2, PPC, D], bf16)
            nc.sync.dma_start(tb[:], bhl[:])
            ta = cpool.tile([128, PPC, NCHUNK, B], f32)
            for p in range(PPC):
                nc.sync.dma_start(ta[:, p], aT[p])

            slices = [(0, 512), (512, 768)]
            for p in range(PPC):
                wt = wpool.tile([128, NCHUNK, D], f32)
                nc.sync.dma_start(wt[:], Wt[p])
                pt = pspool.tile([B, D], f32)
                for (o0, o1) in slices:
                    nc.tensor.matmul(
                        pt[:, o0:o1], ones[:], tb[:, p, o0:o1],
                        start=True, stop=False,
                    )
                for c in range(NCHUNK):
                    for (o0, o1) in slices:
                        nc.tensor.matmul(
                            pt[:, o0:o1], ta[:, p, c, :], wt[:, c, o0:o1],
                            start=False, stop=(c == NCHUNK - 1),
                        )
                ob = opool.tile([B, D], f32)
                nc.vector.tensor_copy(ob[:], pt[:])
                nc.sync.dma_start(out[p], ob[:])

    nc.finalize()
    return nc


def _build_c():
    """Col-tiled patch pairs: two patches share the 128x128 array (cols 0-63 /
    64-127), each streaming its own W. Full-partition PSUM + output DMAs."""
    nc = bacc.Bacc()
    NPAIR = PPC // 2           # 12 pairs + 1 leftover patch
    aT = nc.declare_dram_parameter("aT", [PPC, 128, NCHUNK, B], f32, isOutput=False)
    Wt = nc.declare_dram_parameter("Wt", [PPC, 128, NCHUNK, D], f32, isOutput=False)
    bhl = nc.declare_dram_parameter("bhl", [2, PPC, D], bf16, isOutput=False)
    outp = nc.declare_dram_parameter("outp", [NPAIR, 2 * B, D], f32, isOutput=True)
    outl = nc.declare_dram_parameter("outl", [B, D], f32, isOutput=True)

    with tile.TileContext(nc) as tc:
        with (
            tc.tile_pool(name="const", bufs=1) as cpool,
            tc.tile_pool(name="w", bufs=2) as wpool,
            tc.tile_pool(name="o", bufs=3) as opool,
            tc.tile_pool(name="ps", bufs=2, space="PSUM") as pspool,
        ):
            ones = cpool.tile([2, B], bf16)
            nc.vector.memset(ones[:], 1.0)
            tb = cpool.tile([2, PPC, D], bf16)
            nc.sync.dma_start(tb[:], bhl[:])
            ta = cpool.tile([128, PPC, NCHUNK, B], f32)
            for p in range(PPC):
                nc.sync.dma_start(ta[:, p], aT[p])

            slices = [(0, 512), (512, 768)]
            for j in range(NPAIR):
                p0, p1 = 2 * j, 2 * j + 1
                wt = wpool.tile([128, 2, NCHUNK, D], f32, tag="wt")
                nc.sync.dma_start(
                    wt[:], Wt[p0:p0 + 2].rearrange("p i c o -> i p c o")
                )
                pt = pspool.tile([2 * B, D], f32)
                for (o0, o1) in slices:
                    nc.tensor.matmul(
                        pt[:B, o0:o1], ones[:], tb[:, p0, o0:o1],
                        start=True, stop=False, tile_position=(0, 0),
                    )
                    nc.tensor.matmul(
                        pt[B:, o0:o1], ones[:], tb[:, p1, o0:o1],
                        start=True, stop=False, tile_position=(0, B),
                    )
                for c in range(NCHUNK):
                    for (o0, o1) in slices:
                        nc.tensor.matmul(
                            pt[:B, o0:o1], ta[:, p0, c, :], wt[:, 0, c, o0:o1],
                            start=False, stop=(c == NCHUNK - 1),
                            tile_position=(0, 0),
                        )
                        nc.tensor.matmul(
                            pt[B:, o0:o1], ta[:, p1, c, :], wt[:, 1, c, o0:o1],
                            start=False, stop=(c == NCHUNK - 1),
                            tile_position=(0, B),
                        )
                ob = opool.tile([2 * B, D], f32, tag="ob")
                nc.vector.tensor_copy(ob[:], pt[:])
                nc.sync.dma_start(outp[j], ob[:])

            # leftover patch (PPC is odd)
            p = PPC - 1
            wtl = wpool.tile([128, NCHUNK, D], f32, tag="wtl")
            nc.sync.dma_start(wtl[:], Wt[p])
            ptl = pspool.tile([B, D], f32, tag="ptl")
            for (o0, o1) in slices:
                nc.tensor.matmul(
                    ptl[:, o0:o1], ones[:], tb[:, p, o0:o1],
                    start=True, stop=False,
                )
            for c in range(NCHUNK):
                for (o0, o1) in slices:
                    nc.tensor.matmul(
                        ptl[:, o0:o1], ta[:, p, c, :], wtl[:, c, o0:o1],
                        start=False, stop=(c == NCHUNK - 1),
                    )
            obl = opool.tile([B, D], f32, tag="obl")
            nc.vector.tensor_copy(obl[:], ptl[:])
            nc.sync.dma_start(outl[:], obl[:])

    nc.finalize()
    return nc


def _build_d():
    """Mode C + per-patch W tiles (bufs=5, more outstanding DMAs), chunked aT
    preload, psum bufs=3."""
    nc = bacc.Bacc()
    NPAIR = PPC // 2
    aT = nc.declare_dram_parameter("aT", [PPC, 128, NCHUNK, B], f32, isOutput=False)
    Wt = nc.declare_dram_parameter("Wt", [PPC, 128, NCHUNK, D], f32, isOutput=False)
    bhl = nc.declare_dram_parameter("bhl", [2, PPC, D], bf16, isOutput=False)
    outp = nc.declare_dram_parameter("outp", [NPAIR, 2 * B, D], f32, isOutput=True)
    outl = nc.declare_dram_parameter("outl", [B, D], f32, isOutput=True)

    with tile.TileContext(nc) as tc:
        with (
            tc.tile_pool(name="const", bufs=1) as cpool,
            tc.tile_pool(name="w", bufs=5) as wpool,
            tc.tile_pool(name="o", bufs=3) as opool,
            tc.tile_pool(name="ps", bufs=3, space="PSUM") as pspool,
        ):
            ones = cpool.tile([2, B], bf16)
            nc.vector.memset(ones[:], 1.0)
            tb = cpool.tile([2, PPC, D], bf16)
            nc.sync.dma_start(tb[:], bhl[:])
            ta = cpool.tile([128, PPC, NCHUNK, B], f32)
            for p0 in range(0, PPC, 4):
                p1 = min(p0 + 4, PPC)
                nc.sync.dma_start(
                    ta[:, p0:p1], aT[p0:p1].rearrange("p i c b -> i p c b")
                )

            slices = [(0, 512), (512, 768)]

            def wtile(p):
                t = wpool.tile([128, NCHUNK, D], f32, tag="wt")
                nc.sync.dma_start(t[:], Wt[p])
                return t

            for j in range(NPAIR):
                p0, p1 = 2 * j, 2 * j + 1
                wt0 = wtile(p0)
                wt1 = wtile(p1)
                pt = pspool.tile([2 * B, D], f32)
                for (o0, o1) in slices:
                    nc.tensor.matmul(
                        pt[:B, o0:o1], ones[:], tb[:, p0, o0:o1],
                        start=True, stop=False, tile_position=(0, 0),
                    )
                    nc.tensor.matmul(
                        pt[B:, o0:o1], ones[:], tb[:, p1, o0:o1],
                        start=True, stop=False, tile_position=(0, B),
                    )
                for c in range(NCHUNK):
                    for (o0, o1) in slices:
                        nc.tensor.matmul(
                            pt[:B, o0:o1], ta[:, p0, c, :], wt0[:, c, o0:o1],
                            start=False, stop=(c == NCHUNK - 1),
                            tile_position=(0, 0),
                        )
                        nc.tensor.matmul(
                            pt[B:, o0:o1], ta[:, p1, c, :], wt1[:, c, o0:o1],
                            start=False, stop=(c == NCHUNK - 1),
                            tile_position=(0, B),
                        )
                ob = opool.tile([2 * B, D], f32, tag="ob")
                nc.vector.tensor_copy(ob[:], pt[:])
                nc.sync.dma_start(outp[j], ob[:])

            # leftover patch (PPC is odd)
            p = PPC - 1
            wtl = wtile(p)
            ptl = pspool.tile([B, D], f32, tag="pt")
            for (o0, o1) in slices:
                nc.tensor.matmul(
                    ptl[:, o0:o1], ones[:], tb[:, p, o0:o1],
                    start=True, stop=False,
                )
            for c in range(NCHUNK):
                for (o0, o1) in slices:
                    nc.tensor.matmul(
                        ptl[:, o0:o1], ta[:, p, c, :], wtl[:, c, o0:o1],
                        start=False, stop=(c == NCHUNK - 1),
                    )
            obl = opool.tile([B, D], f32, tag="obl")
            nc.vector.tensor_copy(obl[:], ptl[:])
            nc.sync.dma_start(outl[:], obl[:])

    nc.finalize()
    return nc


def _build_e():
    """Mode D + DMA spread across engines: W loads alternate between the SP
    and ACT HWDGE rings, aT/bias/output DMAs ride SWDGE (gpsimd). Leftover
    solo patch runs first so the kernel ends on a full pair."""
    nc = bacc.Bacc()
    NPAIR = PPC // 2
    aT = nc.declare_dram_parameter("aT", [PPC, 128, NCHUNK, B], f32, isOutput=False)
    Wt = nc.declare_dram_parameter("Wt", [PPC, 128, NCHUNK, D], f32, isOutput=False)
    bhl = nc.declare_dram_parameter("bhl", [2, PPC, D], bf16, isOutput=False)
    outp = nc.declare_dram_parameter("outp", [NPAIR, 2 * B, D], f32, isOutput=True)
    outl = nc.declare_dram_parameter("outl", [B, D], f32, isOutput=True)

    with tile.TileContext(nc) as tc:
        with (
            tc.tile_pool(name="const", bufs=1) as cpool,
            tc.tile_pool(name="w", bufs=5) as wpool,
            tc.tile_pool(name="o", bufs=3) as opool,
            tc.tile_pool(name="ps", bufs=3, space="PSUM") as pspool,
        ):
            ones = cpool.tile([2, B], bf16)
            nc.vector.memset(ones[:], 1.0)
            tb = cpool.tile([2, PPC, D], bf16)
            nc.gpsimd.dma_start(tb[:], bhl[:])
            ta = cpool.tile([128, PPC, NCHUNK, B], f32)
            for p0 in range(0, PPC, 4):
                p1 = min(p0 + 4, PPC)
                nc.gpsimd.dma_start(
                    ta[:, p0:p1], aT[p0:p1].rearrange("p i c b -> i p c b")
                )

            slices = [(0, 512), (512, 768)]
            _wcnt = [0]

            def wtile(p):
                t = wpool.tile([128, NCHUNK, D], f32, tag="wt")
                eng = nc.sync if _wcnt[0] % 2 == 0 else nc.scalar
                _wcnt[0] += 1
                eng.dma_start(t[:], Wt[p])
                return t

            # leftover solo patch first (PPC is odd)
            p = PPC - 1
            wtl = wtile(p)
            ptl = pspool.tile([B, D], f32, tag="pt")
            for (o0, o1) in slices:
                nc.tensor.matmul(
                    ptl[:, o0:o1], ones[:], tb[:, p, o0:o1],
                    start=True, stop=False,
                )
            for c in range(NCHUNK):
                for (o0, o1) in slices:
                    nc.tensor.matmul(
                        ptl[:, o0:o1], ta[:, p, c, :], wtl[:, c, o0:o1],
                        start=False, stop=(c == NCHUNK - 1),
                    )
            obl = opool.tile([B, D], f32, tag="obl")
            nc.vector.tensor_copy(obl[:], ptl[:])
            nc.gpsimd.dma_start(outl[:], obl[:])

            for j in range(NPAIR):
                p0, p1 = 2 * j, 2 * j + 1
                wt0 = wtile(p0)
                wt1 = wtile(p1)
                pt = pspool.tile([2 * B, D], f32, tag="pt")
                for (o0, o1) in slices:
                    nc.tensor.matmul(
                        pt[:B, o0:o1], ones[:], tb[:, p0, o0:o1],
                        start=True, stop=False, tile_position=(0, 0),
                    )
                    nc.tensor.matmul(
                        pt[B:, o0:o1], ones[:], tb[:, p1, o0:o1],
                        start=True, stop=False, tile_position=(0, B),
                    )
                for c in range(NCHUNK):
                    for (o0, o1) in slices:
                        nc.tensor.matmul(
                            pt[:B, o0:o1], ta[:, p0, c, :], wt0[:, c, o0:o1],
                            start=False, stop=(c == NCHUNK - 1),
                            tile_position=(0, 0),
                        )
                        nc.tensor.matmul(
                            pt[B:, o0:o1], ta[:, p1, c, :], wt1[:, c, o0:o1],
                            start=False, stop=(c == NCHUNK - 1),
                            tile_position=(0, B),
                        )
                ob = opool.tile([2 * B, D], f32, tag="ob")
                nc.vector.tensor_copy(ob[:], pt[:])
                nc.gpsimd.dma_start(outp[j], ob[:])

    nc.finalize()
    return nc


def _build_d2():
    """Mode D + solo patch first; W stream pure on the SP HWDGE ring; aT/bias/
    output DMAs on the ACT HWDGE ring; deeper W prefetch."""
    nc = bacc.Bacc()
    NPAIR = PPC // 2
    aT = nc.declare_dram_parameter("aT", [PPC, 128, NCHUNK, B], f32, isOutput=False)
    Wt = nc.declare_dram_parameter("Wt", [PPC, 128, NCHUNK, D], f32, isOutput=False)
    bhl = nc.declare_dram_parameter("bhl", [2, PPC, D], bf16, isOutput=False)
    outp = nc.declare_dram_parameter("outp", [NPAIR, 2 * B, D], f32, isOutput=True)
    outl = nc.declare_dram_parameter("outl", [B, D], f32, isOutput=True)

    with tile.TileContext(nc) as tc:
        with (
            tc.tile_pool(name="const", bufs=1) as cpool,
            tc.tile_pool(name="w", bufs=6) as wpool,
            tc.tile_pool(name="o", bufs=3) as opool,
            tc.tile_pool(name="ps", bufs=3, space="PSUM") as pspool,
        ):
            ones = cpool.tile([2, B], bf16)
            nc.vector.memset(ones[:], 1.0)
            tb = cpool.tile([2, PPC, D], bf16)
            nc.scalar.dma_start(tb[:], bhl[:])
            ta = cpool.tile([128, PPC, NCHUNK, B], f32)
            for p0 in range(0, PPC, 4):
                p1 = min(p0 + 4, PPC)
                nc.scalar.dma_start(
                    ta[:, p0:p1], aT[p0:p1].rearrange("p i c b -> i p c b")
                )

            slices = [(0, 512), (512, 768)]
            _wcnt = [0]

            def wtile(p):
                t = wpool.tile([128, NCHUNK, D], f32, tag="wt")
                _wcnt[0] += 1
                nc.sync.dma_start(t[:], Wt[p])
                return t

            # leftover solo patch first (PPC is odd)
            p = PPC - 1
            wtl = wtile(p)
            ptl = pspool.tile([B, D], f32, tag="pt")
            for (o0, o1) in slices:
                nc.tensor.matmul(
                    ptl[:, o0:o1], ones[:], tb[:, p, o0:o1],
                    start=True, stop=False,
                )
            for c in range(NCHUNK):
                for (o0, o1) in slices:
                    nc.tensor.matmul(
                        ptl[:, o0:o1], ta[:, p, c, :], wtl[:, c, o0:o1],
                        start=False, stop=(c == NCHUNK - 1),
                    )
            obl = opool.tile([B, D], f32, tag="obl")
            nc.vector.tensor_copy(obl[:], ptl[:])
            nc.scalar.dma_start(outl[:], obl[:])

            for j in range(NPAIR):
                p0, p1 = 2 * j, 2 * j + 1
                wt0 = wtile(p0)
                wt1 = wtile(p1)
                pt = pspool.tile([2 * B, D], f32, tag="pt")
                for (o0, o1) in slices:
                    nc.tensor.matmul(
                        pt[:B, o0:o1], ones[:], tb[:, p0, o0:o1],
                        start=True, stop=False, tile_position=(0, 0),
                    )
                    nc.tensor.matmul(
                        pt[B:, o0:o1], ones[:], tb[:, p1, o0:o1],
                        start=True, stop=False, tile_position=(0, B),
                    )
                for c in range(NCHUNK):
                    for (o0, o1) in slices:
                        nc.tensor.matmul(
                            pt[:B, o0:o1], ta[:, p0, c, :], wt0[:, c, o0:o1],
                            start=False, stop=(c == NCHUNK - 1),
                            tile_position=(0, 0),
                        )
                        nc.tensor.matmul(
                            pt[B:, o0:o1], ta[:, p1, c, :], wt1[:, c, o0:o1],
                            start=False, stop=(c == NCHUNK - 1),
                            tile_position=(0, B),
                        )
                ob = opool.tile([2 * B, D], f32, tag="ob")
                nc.vector.tensor_copy(ob[:], pt[:])
                nc.scalar.dma_start(outp[j], ob[:])

    nc.finalize()
    return nc


def _build_d3():
    """D2 + each W patch load split into two halves so PE starts on the first
    half while the second streams in."""
    nc = bacc.Bacc()
    NPAIR = PPC // 2
    aT = nc.declare_dram_parameter("aT", [PPC, 128, NCHUNK, B], f32, isOutput=False)
    Wt = nc.declare_dram_parameter("Wt", [PPC, 128, NCHUNK, D], f32, isOutput=False)
    bhl = nc.declare_dram_parameter("bhl", [2, PPC, D], bf16, isOutput=False)
    outp = nc.declare_dram_parameter("outp", [NPAIR, 2 * B, D], f32, isOutput=True)
    outl = nc.declare_dram_parameter("outl", [B, D], f32, isOutput=True)

    with tile.TileContext(nc) as tc:
        with (
            tc.tile_pool(name="const", bufs=1) as cpool,
            tc.tile_pool(name="w", bufs=6) as wpool,
            tc.tile_pool(name="o", bufs=3) as opool,
            tc.tile_pool(name="ps", bufs=3, space="PSUM") as pspool,
        ):
            ones = cpool.tile([2, B], bf16)
            nc.vector.memset(ones[:], 1.0)
            tb = cpool.tile([2, PPC, D], bf16)
            nc.scalar.dma_start(tb[:], bhl[:])
            ta = cpool.tile([128, PPC, NCHUNK, B], f32)
            for p0 in range(0, PPC, 4):
                p1 = min(p0 + 4, PPC)
                nc.scalar.dma_start(
                    ta[:, p0:p1], aT[p0:p1].rearrange("p i c b -> i p c b")
                )

            slices = [(0, 512), (512, 768)]
            _wcnt = [0]

            def wtile(p):
                t = wpool.tile([128, NCHUNK, D], f32, tag="wt")
                _wcnt[0] += 1
                h = NCHUNK // 2
                nc.sync.dma_start(t[:, :h], Wt[p, :, :h])
                nc.sync.dma_start(t[:, h:], Wt[p, :, h:])
                return t

            # leftover solo patch first (PPC is odd)
            p = PPC - 1
            wtl = wtile(p)
            ptl = pspool.tile([B, D], f32, tag="pt")
            for (o0, o1) in slices:
                nc.tensor.matmul(
                    ptl[:, o0:o1], ones[:], tb[:, p, o0:o1],
                    start=True, stop=False,
                )
            for c in range(NCHUNK):
                for (o0, o1) in slices:
                    nc.tensor.matmul(
                        ptl[:, o0:o1], ta[:, p, c, :], wtl[:, c, o0:o1],
                        start=False, stop=(c == NCHUNK - 1),
                    )
            obl = opool.tile([B, D], f32, tag="obl")
            nc.vector.tensor_copy(obl[:], ptl[:])
            nc.scalar.dma_start(outl[:], obl[:])

            for j in range(NPAIR):
                p0, p1 = 2 * j, 2 * j + 1
                wt0 = wtile(p0)
                wt1 = wtile(p1)
                pt = pspool.tile([2 * B, D], f32, tag="pt")
                for (o0, o1) in slices:
                    nc.tensor.matmul(
                        pt[:B, o0:o1], ones[:], tb[:, p0, o0:o1],
                        start=True, stop=False, tile_position=(0, 0),
                    )
                    nc.tensor.matmul(
                        pt[B:, o0:o1], ones[:], tb[:, p1, o0:o1],
                        start=True, stop=False, tile_position=(0, B),
                    )
                for c in range(NCHUNK):
                    for (o0, o1) in slices:
                        nc.tensor.matmul(
                            pt[:B, o0:o1], ta[:, p0, c, :], wt0[:, c, o0:o1],
                            start=False, stop=(c == NCHUNK - 1),
                            tile_position=(0, 0),
                        )
                        nc.tensor.matmul(
                            pt[B:, o0:o1], ta[:, p1, c, :], wt1[:, c, o0:o1],
                            start=False, stop=(c == NCHUNK - 1),
                            tile_position=(0, B),
                        )
                ob = opool.tile([2 * B, D], f32, tag="ob")
                nc.vector.tensor_copy(ob[:], pt[:])
                nc.scalar.dma_start(outp[j], ob[:])

    nc.finalize()
    return nc


def _patchify(x):
    # [B, C, H, W] -> [B, 196, 768] in MAE ordering (n c h p w q -> n h w p q c)
    Bn, C, H, Wd = x.shape
    h = H // 16
    xr = x.reshape(Bn, C, h, 16, h, 16)
    xr = np.transpose(xr, (0, 2, 4, 3, 5, 1))
    return xr.reshape(Bn, h * h, 16 * 16 * C)


def kernel(x, W, b, _trace=False, _mode=None):
    global LAST_RESULTS
    mode = _mode or os.environ.get("KERNEL_MODE", "A")

    x = np.asarray(x, dtype=np.float32)
    W = np.asarray(W, dtype=np.float32)
    b = np.asarray(b, dtype=np.float32)

    patches = _patchify(x)                      # [64, 196, 768]

    in_maps = []
    for k in range(N_CORES):
        lo = k * PPC
        idx = np.arange(lo, lo + PPC)
        idx[idx >= NP] = 0                      # pad tail with patch 0
        psl = patches[:, idx, :]                # [64, 25, 768]
        wsl = W[idx]                            # [25, 768, 768]
        bsl = b[idx]                            # [25, 768]

        if mode in ("A2", "C", "D", "E", "D2", "D3"):
            aT = np.ascontiguousarray(
                psl.transpose(2, 1, 0)          # [768, 25, 64]
                .reshape(NCHUNK, 128, PPC, B)
                .transpose(2, 1, 0, 3)          # [25, 128, 6, 64]
            )
        else:
            aT = np.ascontiguousarray(
                psl.transpose(2, 1, 0)          # [768, 25, 64]
                .reshape(NCHUNK, 128, PPC, B)
                .transpose(1, 2, 0, 3)          # [128, 25, 6, 64]
            )
        Wt = np.ascontiguousarray(
            wsl.transpose(0, 2, 1)              # [25, 768(i), 768(o)]
            .reshape(PPC, NCHUNK, 128, D)
            .transpose(0, 2, 1, 3)              # [25, 128, 6, 768]
        )
        m = {"aT": aT, "Wt": Wt}
        if mode == "A":
            hi = bsl.astype(ml_dtypes.bfloat16)
            lo_ = (bsl - hi.astype(np.float32)).astype(ml_dtypes.bfloat16)
            m["bhl"] = np.ascontiguousarray(np.stack([hi, lo_], axis=0))
        elif mode in ("A2", "C", "D", "E", "D2", "D3"):
            hi = bsl.astype(ml_dtypes.bfloat16)
            lo_ = (bsl - hi.astype(np.float32)).astype(ml_dtypes.bfloat16)
            m["bhl"] = np.ascontiguousarray(np.stack([hi, lo_], axis=0))
        else:
            m["bpp"] = np.ascontiguousarray(
                bsl.reshape(PPC, NCHUNK, 128).transpose(2, 0, 1)
            )
        in_maps.append(m)

    key = mode
    if key not in _NC_CACHE:
        _NC_CACHE[key] = _build(mode)
    nc = _NC_CACHE[key]

    res = run_bass_kernel_spmd(nc, in_maps, list(range(N_CORES)), trace=_trace)
    LAST_RESULTS = res

    if mode in ("C", "D", "E", "D2", "D3"):
        parts = np.concatenate(
            [
                np.concatenate(
                    [
                        res.results[k]["outp"].reshape(PPC - 1, B, D),
                        res.results[k]["outl"][None],
                    ],
                    axis=0,
                )[None]
                for k in range(N_CORES)
            ]
        )                                       # [8, 25, 64, 768]
        full = parts.transpose(2, 0, 1, 3).reshape(B, N_CORES * PPC, D)
    else:
        parts = np.stack([res.results[k]["out"] for k in range(N_CORES)])
        if mode in ("A", "A2"):
            # parts [8, 25, 64, 768] -> [64, 200, 768]
            full = parts.transpose(2, 0, 1, 3).reshape(B, N_CORES * PPC, D)
        else:
            # parts [8, 25, 128(o_in), 6(oc), 64(b)] -> [64, 200, 768]
            full = parts.transpose(4, 0, 1, 3, 2).reshape(B, N_CORES * PPC, D)
    return np.ascontiguousarray(full[:, :NP, :])
